# revision 1
# baseline (speedup 1.0000x reference)
"""Self-contained TRN2 Bass kernel for nn_MultiHeadAttentionLayer
(GNN multi-head attention message passing), 8 NeuronCores.

kernel(**inputs) takes the FULL unsharded inputs (h, Wq, bq, Wk, bk, Wv,
bv, src, dst) as numpy arrays and returns the FULL [N, H, D] float32
output. Sharding: edges are partitioned by dst range across the 8 cores
(no collectives needed); each core projects K/V for all nodes, gathers
K|V rows per edge with dma_gather, computes scores/softmax weights on
DVE/ACT, and segment-sums into its private dst slice via one-hot
matmuls on the TensorEngine.
"""

from dataclasses import dataclass, field

import numpy as np
import ml_dtypes

import concourse.bass as bass
import concourse.tile as tile
from concourse import bacc, mybir
from concourse.bass import ts
from concourse.bass_utils import run_bass_kernel_spmd

BF16 = ml_dtypes.bfloat16
F32 = np.float32
AF = mybir.ActivationFunctionType
ALU = mybir.AluOpType

CH_CAP = 18          # max blocks per gather/compute chunk


@dataclass
class Cfg:
    N: int
    IN: int
    H: int
    D: int
    n_cores: int = 8
    NPC: int = 0
    NT: int = 0
    NSB: int = 0
    CHL: list = field(default_factory=list)   # lo-group blocks per sb
    CHH: list = field(default_factory=list)   # hi-group blocks per sb
    ASSIGN: list = field(default_factory=list)  # [core][pos] -> global sb

    @property
    def C(self):
        return self.H * self.D

    @property
    def KA(self):
        return self.IN // 128

    @property
    def NT2(self):
        return self.NT // 2

    @property
    def SBLK(self):
        return [l + h for l, h in zip(self.CHL, self.CHH)]


def make_cfg(N, IN, H, D, src, dst, n_cores=8):
    cfg = Cfg(N=N, IN=IN, H=H, D=D, n_cores=n_cores)
    cfg.NPC = -(-N // (n_cores * 128)) * 128
    cfg.NT = cfg.NPC * n_cores
    cfg.NSB = cfg.NPC // 128
    src = np.asarray(src)
    dst = np.asarray(dst)
    gsb = dst // 128                       # global super-block of each edge
    nsb_tot = cfg.NSB * n_cores
    is_hi = (src >= cfg.NT2).astype(np.int64)
    counts = np.zeros((nsb_tot, 2), dtype=np.int64)
    np.add.at(counts, (np.minimum(gsb, nsb_tot - 1), is_hi), 1)
    # balanced assignment: position k across cores gets super-blocks of
    # similar total size -> minimal max-over-cores padding
    order = np.argsort(-(counts[:, 0] + counts[:, 1]), kind="stable")
    cfg.ASSIGN = [[int(order[k * n_cores + i]) for k in range(cfg.NSB)]
                  for i in range(n_cores)]
    cfg.CHL = []
    cfg.CHH = []
    for k in range(cfg.NSB):
        grp = order[k * n_cores:(k + 1) * n_cores]
        cfg.CHL.append(max(1, int(-(-counts[grp, 0].max() // 128))))
        cfg.CHH.append(max(1, int(-(-counts[grp, 1].max() // 128))))
    return cfg


def chunk_plan(cfg):
    """[(j, g, cb, CH, key)] in canonical order; key indexes nval columns."""
    plan = []
    key = 0
    for j in range(cfg.NSB):
        for g, CHG in ((0, cfg.CHL[j]), (1, cfg.CHH[j])):
            cap = CH_CAP if not (g == 1 and j == cfg.NSB - 1) \
                else max(6, -(-CHG // 3))
            b0 = 0
            while b0 < CHG:
                plan.append((j, g, b0, min(cap, CHG - b0), key))
                key += 1
                b0 += cap
    return plan


def _wrap16(idx, epb):
    base = idx.reshape(epb // 16, 16).T.astype(np.int16)
    return np.tile(base, (8, 1))


def prep(cfg: Cfg, h, Wq, bq, Wk, bk, Wv, bv, src, dst):
    N, IN, H, D, C = cfg.N, cfg.IN, cfg.H, cfg.D, cfg.C
    scale = 1.0 / np.sqrt(np.float32(D))

    hT = np.zeros((IN, cfg.NT), dtype=BF16)
    hT[:, :N] = np.asarray(h).T.astype(BF16)
    wkv = np.concatenate([np.asarray(Wk), np.asarray(Wv)], axis=1).astype(BF16)
    bkv = np.concatenate([np.asarray(bk), np.asarray(bv)])[None, :].astype(BF16)
    wq = (np.asarray(Wq) * scale).astype(BF16)
    bqs = (np.asarray(bq) * scale)[None, :].astype(BF16)

    src = np.asarray(src).astype(np.int64)
    dst = np.asarray(dst).astype(np.int64)

    sum_blk = sum(cfg.SBLK)
    sum_epb = sum_blk * 128
    marange = np.arange(128, dtype=np.int64)

    gsb_of = dst // 128
    in_maps = []
    for i in range(cfg.n_cores):
        srcidx = np.zeros(sum_epb, dtype=np.int64)
        ld = np.full((sum_blk, 128), 255, dtype=np.int64)
        off_e = 0
        off_b = 0
        for j in range(cfg.NSB):
            g_sb = cfg.ASSIGN[i][j]
            insb = gsb_of == g_sb
            es, ed = src[insb], dst[insb] - g_sb * 128
            for g, chg in ((0, cfg.CHL[j]), (1, cfg.CHH[j])):
                if g == 0:
                    gsel = es < cfg.NT2
                    gidx = es[gsel]
                else:
                    gsel = es >= cfg.NT2
                    gidx = es[gsel] - cfg.NT2
                cnt = gidx.shape[0]
                epb = chg * 128
                assert cnt <= epb, (i, j, g, cnt, epb)
                srcidx[off_e:off_e + cnt] = gidx
                ldj = np.full(epb, 255, dtype=np.int64)
                ldj[:cnt] = ed[gsel]
                ld[off_b:off_b + chg, :] = ldj.reshape(chg, 128)
                off_e += epb
                off_b += chg

        srcw_parts = []
        off = 0
        for j in range(cfg.NSB):
            for chg in (cfg.CHL[j], cfg.CHH[j]):
                epb = chg * 128
                srcw_parts.append(_wrap16(srcidx[off:off + epb], epb))
                off += epb
        srcw = np.concatenate(srcw_parts, axis=1)

        onehot = (ld[:, :, None] == marange[None, None, :])       # [bb, e, m]
        Sh = np.ascontiguousarray(onehot.transpose(1, 0, 2)).astype(BF16)
        ShT = np.ascontiguousarray(onehot.transpose(2, 0, 1)).astype(BF16)

        cols = np.concatenate(
            [np.arange(cfg.ASSIGN[i][j] * 128, cfg.ASSIGN[i][j] * 128 + 128)
             for j in range(cfg.NSB)])
        in_maps.append({
            "hT": hT,
            "hTq": np.ascontiguousarray(hT[:, cols]),
            "wkv": wkv, "bkv": bkv, "wq": wq, "bq": bqs,
            "srcidx": srcw,
            "Sh": Sh, "ShT": ShT,
        })
    return in_maps


def build(cfg: Cfg):
    N, IN, H, D, C = cfg.N, cfg.IN, cfg.H, cfg.D, cfg.C
    KA = cfg.KA
    C2 = 2 * C
    CZ = C + H
    sum_blk = sum(cfg.SBLK)
    sum_epb = sum_blk * 128
    bf = mybir.dt.bfloat16
    f32 = mybir.dt.float32

    nc = bacc.Bacc("TRN2", target_bir_lowering=False, debug=False)
    hT = nc.dram_tensor("hT", [IN, cfg.NT], bf, kind="ExternalInput").ap()
    hTq = nc.dram_tensor("hTq", [IN, cfg.NPC], bf, kind="ExternalInput").ap()
    wkv = nc.dram_tensor("wkv", [IN, C2], bf, kind="ExternalInput").ap()
    bkv = nc.dram_tensor("bkv", [1, C2], bf, kind="ExternalInput").ap()
    wq = nc.dram_tensor("wq", [IN, C], bf, kind="ExternalInput").ap()
    bq = nc.dram_tensor("bq", [1, C], bf, kind="ExternalInput").ap()
    srcidx = nc.dram_tensor("srcidx", [128, sum_epb // 16], mybir.dt.int16,
                            kind="ExternalInput").ap()
    Sh_d = nc.dram_tensor("Sh", [128, sum_blk, 128], bf, kind="ExternalInput").ap()
    ShT_d = nc.dram_tensor("ShT", [128, sum_blk, 128], bf, kind="ExternalInput").ap()
    out = nc.dram_tensor("out", [cfg.NPC, C], f32, kind="ExternalOutput").ap()

    with tile.TileContext(nc) as tc:
        with (
            tc.tile_pool(name="dram", bufs=1, space="DRAM") as dramp,
            tc.tile_pool(name="const", bufs=1) as constp,
        ):
            kv_lo = dramp.tile([cfg.NT2, C2], bf)
            kv_hi = dramp.tile([cfg.NT2, C2], bf)

            wkvt = constp.tile([128, KA, C2], bf)
            nc.sync.dma_start(wkvt[:], wkv.rearrange("(a p) c -> p a c", p=128))
            wqt = constp.tile([128, KA, C], bf)
            nc.sync.dma_start(wqt[:], wq.rearrange("(a p) c -> p a c", p=128))
            bkvt = constp.tile([1, C2], bf)
            nc.sync.dma_start(bkvt[:], bkv[:])
            bqt = constp.tile([1, C], bf)
            nc.sync.dma_start(bqt[:], bq[:])
            ones1 = constp.tile([1, 128], bf)
            nc.vector.memset(ones1[:], 1.0)
            srct = constp.tile([128, sum_epb // 16], mybir.dt.int16)
            nc.sync.dma_start(srct[:], srcidx[:])
            qs = constp.tile([128, cfg.NSB, C], bf)
            bias_kv = constp.tile([128, C2], bf)


            # ---------------- Phase A ----------------
            import contextlib
            pg_ctx = contextlib.ExitStack()
            pg = pg_ctx.enter_context(tc.tile_pool(name="pb_g", bufs=5))
            with (
                tc.tile_pool(name="pa_h", bufs=1) as pah,
                tc.tile_pool(name="pa_ps", bufs=6, space="PSUM") as paps,
                tc.tile_pool(name="pa_bps", bufs=1, space="PSUM") as pabps,
                tc.tile_pool(name="pa_sb", bufs=4) as pasb,
            ):
                bps = pabps.tile([128, C2], f32, tag="bps")
                nc.tensor.matmul(out=bps[:], lhsT=ones1[:], rhs=bkvt[:],
                                 start=True, stop=True)
                nc.vector.tensor_copy(bias_kv[:], bps[:])
                bpq = pabps.tile([128, C], f32, tag="bpq")
                nc.tensor.matmul(out=bpq[:], lhsT=ones1[:], rhs=bqt[:],
                                 start=True, stop=True)
                bias_q = pasb.tile([128, C], bf, tag="biasq")
                nc.vector.tensor_copy(bias_q[:], bpq[:])

                hts = pah.tile([128, KA, cfg.NT], bf)
                hT_r = hT.rearrange("(a p) n -> p a n", p=128)
                NSPL = 8
                SPL = cfg.NT // NSPL
                for sp in range(NSPL):
                    nc.sync.dma_start(hts[:, :, ts(sp, SPL)], hT_r[:, :, ts(sp, SPL)])
                htq = pah.tile([128, KA, cfg.NPC], bf)
                nc.sync.dma_start(htq[:], hTq.rearrange("(a p) n -> p a n", p=128))

                NC2 = cfg.NT2 // 128

                def kv_chunk(cc, tbl):
                    ps = paps.tile([128, C2], f32, tag="psA")
                    for a in range(KA):
                        nc.tensor.matmul(out=ps[:], lhsT=hts[:, a, ts(cc, 128)],
                                         rhs=wkvt[:, a, :], start=(a == 0),
                                         stop=(a == KA - 1))
                    buf = pasb.tile([128, C2], bf, tag="bufA")
                    nc.scalar.copy(buf[:, 0:C], ps[:, 0:C])
                    nc.vector.tensor_tensor(buf[:, C:C2], ps[:, C:C2],
                                            bias_kv[:, C:C2], op=ALU.add)
                    nc.sync.dma_start(tbl[ts(cc % NC2, 128), :], buf[:])

                for cc in range(NC2):
                    kv_chunk(cc, kv_lo[:])

                for qc in range(cfg.NSB):
                    psq = paps.tile([128, C], f32, tag="psA", name="psq")
                    for a in range(KA):
                        nc.tensor.matmul(out=psq[:], lhsT=htq[:, a, ts(qc, 128)],
                                         rhs=wqt[:, a, :], start=(a == 0),
                                         stop=(a == KA - 1))
                    nc.vector.tensor_tensor(qs[:, qc, :], psq[:], bias_q[:],
                                            op=ALU.add)

                for cc in range(NC2, 2 * NC2):
                    kv_chunk(cc, kv_hi[:])

            # ---------------- Phase B ----------------
            grp_off = {}
            off_b = 0
            for j in range(cfg.NSB):
                grp_off[(j, 0)] = off_b
                off_b += cfg.CHL[j]
                grp_off[(j, 1)] = off_b
                off_b += cfg.CHH[j]

            with (
                tc.tile_pool(name="pb_t", bufs=2) as pt,
                tc.tile_pool(name="pb_c", bufs=2) as pc,
                tc.tile_pool(name="pb_w", bufs=2) as pw,
                tc.tile_pool(name="pb_s", bufs=2) as psm,
                tc.tile_pool(name="pb_ps", bufs=4, space="PSUM") as pps,
                tc.tile_pool(name="pb_qps", bufs=4, space="PSUM") as pqps,
            ):
                pswz_of = {}
                chunk_counter = [0]

                plan_all = chunk_plan(cfg)

                def process_group(j, g, tbl, last_of_sb, split_tail=False):
                    gb = grp_off[(j, g)]
                    if j not in pswz_of:
                        pswz_of[j] = pps.tile([128, CZ], f32, tag="pswz", name=f"pswz{j}")
                    pswz = pswz_of[j]
                    first_of_sb = (g == 0)
                    chunks = [(cb, CH, key) for (jj, gg, cb, CH, key) in plan_all
                              if jj == j and gg == g]
                    for (ci, (cb, CH, key)) in enumerate(chunks):
                        cbk = gb + cb
                        ce = cbk * 128
                        EPC = CH * 128
                        kvg = pg.tile([128, CH, C2], bf, tag="kvg")
                        nc.gpsimd.dma_gather(
                            kvg[:], tbl, srct[:, ce // 16:(ce + EPC) // 16],
                            EPC, EPC, C2, single_packet=False)
                        sh = pt.tile([128, CH, 128], bf, tag="sh")
                        nc.scalar.dma_start(sh[:], Sh_d[:, cbk:cbk + CH, :])
                        sht = pt.tile([128, CH, 128], bf, tag="sht")
                        nc.scalar.dma_start(sht[:], ShT_d[:, cbk:cbk + CH, :])

                        qg = pc.tile([128, CH, C], bf, tag="qg")
                        for b0q in range(0, CH, 2):
                            bw = min(2, CH - b0q)
                            qps = pqps.tile([128, 2, C], f32, tag="qps")
                            for b in range(b0q, b0q + bw):
                                nc.tensor.matmul(out=qps[:, b - b0q, :],
                                                 lhsT=sht[:, b, :],
                                                 rhs=qs[:, j, :],
                                                 start=True, stop=True)
                            nc.scalar.copy(qg[:, b0q:b0q + bw, :], qps[:, 0:bw, :])

                        P = pc.tile([128, CH, C], bf, tag="P")
                        nc.vector.tensor_tensor(P[:], kvg[:, :, 0:C], qg[:],
                                                op=ALU.mult)
                        P4 = P[:].rearrange("p b (h d) -> p b h d", d=D)
                        cur = P4
                        w = D
                        while w > 2:
                            w //= 2
                            nxt = pc.tile([128, CH, H, w], bf, tag=f"s{w}")
                            nc.vector.tensor_tensor(
                                nxt[:], cur[:, :, :, 0:w], cur[:, :, :, w:2 * w],
                                op=ALU.add)
                            cur = nxt[:]
                        sc = pc.tile([128, CH, H], f32, tag="sc")
                        nc.vector.tensor_tensor(
                            sc[:].unsqueeze(3), cur[:, :, :, 0:1], cur[:, :, :, 1:2],
                            op=ALU.add)

                        wvz = pw.tile([128, CH, CZ], bf, tag="wvz")
                        nc.scalar.activation(wvz[:, :, C:CZ], sc[:], AF.Exp)
                        nc.scalar.activation(
                            P4, sc[:].unsqueeze(3).broadcast_to([128, CH, H, D]),
                            AF.Exp)
                        nc.vector.tensor_tensor(
                            wvz[:, :, 0:C], kvg[:, :, C:C2], P[:], op=ALU.mult)

                        for b in range(CH):
                            nc.tensor.matmul(
                                out=pswz[:], lhsT=sh[:, b, :], rhs=wvz[:, b, :],
                                start=(first_of_sb and ci == 0 and b == 0),
                                stop=(last_of_sb and ci == len(chunks) - 1
                                      and b == CH - 1))

                def finalize(j):
                    pswz = pswz_of.pop(j)
                    zm = psm.tile([128, H], f32, tag="zm")
                    nc.vector.tensor_scalar(zm[:], pswz[:, C:CZ], 1e-30, None,
                                            op0=ALU.max)
                    zr = psm.tile([128, H], f32, tag="zr")
                    nc.vector.reciprocal(zr[:], zm[:])
                    of = psm.tile([128, C], f32, tag="of")
                    nc.vector.tensor_tensor(
                        of[:].rearrange("p (h d) -> p h d", d=D),
                        pswz[:, 0:C].rearrange("p (h d) -> p h d", d=D),
                        zr[:].unsqueeze(2).broadcast_to([128, H, D]),
                        op=ALU.mult)
                    nc.scalar.dma_start(out[ts(j, 128), :], of[:])

                NLEAD = min(3, cfg.NSB)
                for j in range(NLEAD):
                    process_group(j, 0, kv_lo[:], last_of_sb=False)
                for k in range(cfg.NSB):
                    process_group(k, 1, kv_hi[:], last_of_sb=True,
                                  split_tail=(k == cfg.NSB - 1))
                    finalize(k)
                    if k + NLEAD < cfg.NSB:
                        process_group(k + NLEAD, 0, kv_lo[:], last_of_sb=False)
            pg_ctx.close()

    nc.compile()
    return nc


def run(cfg: Cfg, in_maps, trace=False, nc=None):
    if nc is None:
        nc = build(cfg)
    res = run_bass_kernel_spmd(nc, in_maps, core_ids=list(range(cfg.n_cores)),
                               trace=trace)
    full = np.zeros((cfg.NT, cfg.C), dtype=np.float32)
    for i in range(cfg.n_cores):
        o = res.results[i]["out"]
        for j in range(cfg.NSB):
            g_sb = cfg.ASSIGN[i][j]
            full[g_sb * 128:(g_sb + 1) * 128] = o[j * 128:(j + 1) * 128]
    full = full[:cfg.N]
    return full.reshape(cfg.N, cfg.H, cfg.D), res


_PROBLEM_N = 10000
_PROBLEM_IN = 256
_PROBLEM_H = 8
_PROBLEM_D = 32


def kernel(h, Wq, bq, Wk, bk, Wv, bv, src, dst):
    h = np.asarray(h)
    N, IN = h.shape
    C = np.asarray(Wq).shape[1]
    H, D = _PROBLEM_H, _PROBLEM_D
    if C != H * D:
        D = C // H
    src = np.asarray(src)
    dst = np.asarray(dst)
    cfg = make_cfg(N, IN, H, D, src, dst)
    in_maps = prep(cfg, h, Wq, bq, Wk, bk, Wv, bv, src, dst)
    out, _ = run(cfg, in_maps, trace=False)
    return out.astype(np.float32)

